# revision 31
# baseline (speedup 1.0000x reference)
"""Trainium2 kernel for nn_Circuit_88871463288913.

Circuit (d=2, n=11 wires, D=2048, B=32):
  psi -> H on every wire -> CNOT ladder -> CRX ladder.

Decomposition (bits q0..q10, q10 fastest; major m = q0..q6, minor mu =
q7..q10):
  - H^11 = H7(major) x H4(minor)
  - CNOT ladder = prefix-XOR permutation = PX7(major), then minor map
    mu' = PX4(mu) ^ (q6' ? 1111 : 0)  conditioned on the post-PX7 bit q6'
  - CRX 0..5 act on major only (complex 128x128 product C)
  - CRX 6 = R_g on q7 conditioned on major bit q6' (g)
  - CRX 7..9 act on minor only (complex 16x16 product K)

Device pipeline per core (batch shard: 4 columns/core).  The state is one
128x128 f32 tile; every gate is a PE matmul with the STATE as the
stationary operand (lhsT), so each matmul applies the gate AND flips the
layout between (major | rho) and (rho | major), rho = reim*64 + mu*4 + b.
A bit-rotation relabel (Prot: m' -> m'' = q6'*64 + rest) makes the q6'
conditioning contiguous halves.

  MM1: rhs = (Prot P7 H7)^T                     -> (rho, m'')
  MM2: rhs = (F_h H4 (x) I4 (x) I2)^T, per half -> (m'', rho)
  MM3: complex C via PSUM accumulation with [Cre | -Cim] stacking
                                                -> (rho, m'')
  MM4: rhs = packed-complex (K R_g)^T, per half -> (m'', rho)

Everything angle-dependent is composed on the host in O(128^2).
"""

import numpy as np

_D = 2048
_N = 11
_B = 32
_NC = 8

_LAST = {}  # stash of the most recent BassKernelResults (for test harness)
_PROG = {}  # cached Bass program
_CONST = {}  # cached angle-independent matrices
_FP16 = True  # device dtype for state/weights (PSUM accumulate is always f32)
_RAW = True  # raw-bass program (no TileContext barrier machinery)


def _walsh(nbits):
    n = 1 << nbits
    i = np.arange(n)
    v = i[:, None] & i[None, :]
    v = v ^ (v >> 8)
    v = v ^ (v >> 4)
    v = v ^ (v >> 2)
    v = v ^ (v >> 1)
    return (1.0 - 2.0 * (v & 1)) / np.sqrt(n)


def _prefix_xor(nbits):
    v = np.arange(1 << nbits)
    res = np.zeros_like(v)
    acc = np.zeros_like(v)
    for k in range(nbits):
        bit = (v >> (nbits - 1 - k)) & 1
        acc = acc ^ bit
        res = res | (acc << (nbits - 1 - k))
    return res


def _perm_mat(p):
    n = len(p)
    M = np.zeros((n, n))
    M[p, np.arange(n)] = 1.0
    return M


def _crx(nbits, cpos, tpos, theta):
    n = 1 << nbits
    i = np.arange(n)
    cd = (i >> (nbits - 1 - cpos)) & 1
    a = cd * theta / 2.0
    flip = i ^ (1 << (nbits - 1 - tpos))
    M = np.zeros((n, n), dtype=complex)
    M[i, i] = np.cos(a)
    M[flip, i] = -1j * np.sin(a)
    return M


def _expand_mu(Mmu):  # 16x16 -> 64x64 (tensor with I4 over batch)
    return np.kron(Mmu, np.eye(4))


def _get_const():
    if _CONST:
        return _CONST
    m = np.arange(128)
    rot = ((m & 1) << 6) | (m >> 1)  # m' -> m''
    inv = np.empty(128, dtype=int)
    inv[rot] = m
    Prot = _perm_mat(rot)
    M1 = Prot @ _perm_mat(_prefix_xor(7)) @ _walsh(7)
    PX4 = _prefix_xor(4)
    H4 = _walsh(4)
    G = [_perm_mat(PX4 ^ (15 * h)) @ H4 for h in (0, 1)]
    W1 = [np.kron(np.eye(2), _expand_mu(G[h])) for h in (0, 1)]
    _CONST.update(rot=rot, inv=inv, Prot=Prot, M1=M1, W1=W1)
    return _CONST


def _build_weights(angles):
    """Returns the 8 rhs matrices (f32, 128x128 each), already transposed."""
    cst = _get_const()
    ang = np.asarray(angles, dtype=np.float64)

    C = np.eye(128, dtype=complex)
    for q in range(6):
        C = _crx(7, q, q + 1, ang[q]) @ C
    Cpp = cst["Prot"] @ C @ cst["Prot"].T

    K = np.eye(16, dtype=complex)
    for q in range(7, 10):
        K = _crx(4, q - 7, q - 6, ang[q]) @ K
    W2 = []
    for g in (0, 1):
        i = np.arange(16)
        a = g * ang[6] / 2.0
        R = np.zeros((16, 16), dtype=complex)
        R[i, i] = np.cos(a)
        R[i ^ 8, i] = -1j * np.sin(a)
        Kg = K @ R
        Kr, Ki = _expand_mu(np.real(Kg)), _expand_mu(np.imag(Kg))
        W2.append(np.block([[Kr, -Ki], [Ki, Kr]]))

    Cre, Cim = np.real(Cpp), np.imag(Cpp)
    mats = [
        cst["M1"].T,
        cst["W1"][0].T,
        cst["W1"][1].T,
        Cre.T,
        Cim.T,
        (-Cim).T,
        W2[0].T,
        W2[1].T,
    ]
    return [np.ascontiguousarray(x, dtype=np.float32) for x in mats]


def _get_program_raw():
    """Raw-bass (no TileContext) version: manual semaphores, no tail
    barrier machinery.  Dataflow is a short serial chain, so manual sync
    is tractable: PE waits on DVE copies, DVE waits on PE matmuls."""
    if "nc" in _PROG:
        return _PROG["nc"]

    import concourse.bacc as bacc
    import concourse.mybir as mybir

    f32 = mybir.dt.float32
    dt = mybir.dt.float16 if _FP16 else mybir.dt.float32
    NW = 9
    BLOB = NW * 128

    nc = bacc.Bacc("TRN2", target_bir_lowering=False)
    blob = nc.declare_dram_parameter("blob", [128, BLOB], dt, isOutput=False)
    o = nc.declare_dram_parameter("o", [128, 128], dt, isOutput=True)

    with (
        nc.sbuf_tensor([128, BLOB], dt) as b_sb,
        nc.sbuf_tensor([128, 128], dt) as s1,
        nc.sbuf_tensor([128, 128], dt) as s2,
        nc.sbuf_tensor([128, 128], dt) as s3,
        nc.sbuf_tensor([128, 128], dt) as s4,
        nc.psum_tensor([128, 128], f32) as p1,
        nc.psum_tensor([128, 128], f32) as p2,
        nc.psum_tensor([128, 128], f32) as p3,
        nc.psum_tensor([128, 128], f32) as p4,
        nc.semaphore() as sd,
        nc.semaphore() as sa,
        nc.semaphore() as sg,
        nc.semaphore() as sh,
        nc.semaphore() as so,
        nc.semaphore() as spe,
        nc.semaphore() as sv,
    ):
        sl = lambda k: b_sb[:, k * 128 : (k + 1) * 128]
        A0, M1T, W10T, W11T, CreT, CimT, nCimT, W20T, W21T = (
            sl(k) for k in range(9)
        )

        # No nc.Block(): instructions are emitted straight into `main`.
        # SWDGE (gpsimd) coalesces packets ~10x better than the HW-DGE
        # rings for these shapes, so all input phases go through it, each
        # with its own semaphore (SW queues may complete out of order).
        hoist = [
            nc.gpsimd.dma_start(b_sb[:, 0:256], blob[:, 0:256]).then_inc(sd, 16),
            nc.scalar.dma_start(b_sb[:, 256:512], blob[:, 256:512]).then_inc(sa, 16),
            nc.sync.dma_start(b_sb[:, 512:896], blob[:, 512:896]).then_inc(sg, 16),
            nc.scalar.dma_start(b_sb[:, 896:1152], blob[:, 896:1152]).then_inc(sh, 16),
        ]

        # PE chain
        nc.tensor.wait_ge(sd, 16)
        nc.tensor.matmul(p1[:], A0, M1T, start=True, stop=True).then_inc(spe, 1)
        nc.tensor.wait_ge(sa, 16)
        nc.tensor.wait_ge(sv, 1)
        nc.tensor.matmul(p2[0:64, :], s1[:, 0:64], W10T, start=True, stop=True)
        nc.tensor.wait_ge(sv, 2)
        nc.tensor.matmul(
            p2[64:128, :], s1[:, 64:128], W11T, start=True, stop=True
        ).then_inc(spe, 1)
        nc.tensor.wait_ge(sg, 16)
        nc.tensor.wait_ge(sv, 3)
        nc.tensor.matmul(p3[0:64, :], s2[:, 0:64], CreT, start=True, stop=False)
        nc.tensor.wait_ge(sv, 4)
        nc.tensor.matmul(p3[0:64, :], s2[:, 64:128], nCimT, start=False, stop=True)
        nc.tensor.matmul(p3[64:128, :], s2[:, 0:64], CimT, start=True, stop=False)
        nc.tensor.matmul(
            p3[64:128, :], s2[:, 64:128], CreT, start=False, stop=True
        ).then_inc(spe, 1)
        nc.tensor.wait_ge(sh, 16)
        nc.tensor.wait_ge(sv, 5)
        nc.tensor.matmul(p4[0:64, :], s3[:, 0:64], W20T, start=True, stop=True)
        nc.tensor.wait_ge(sv, 6)
        nc.tensor.matmul(
            p4[64:128, :], s3[:, 64:128], W21T, start=True, stop=True
        ).then_inc(spe, 1)

        # DVE chain (PSUM -> SBUF bounces), split in free-dim halves so the
        # next PE stage starts as soon as its lhsT half is converted
        nc.vector.wait_ge(spe, 1)
        nc.vector.tensor_copy(s1[:, 0:64], p1[:, 0:64]).then_inc(sv, 1)
        nc.vector.tensor_copy(s1[:, 64:128], p1[:, 64:128]).then_inc(sv, 1)
        nc.vector.wait_ge(spe, 2)
        nc.vector.tensor_copy(s2[:, 0:64], p2[:, 0:64]).then_inc(sv, 1)
        nc.vector.tensor_copy(s2[:, 64:128], p2[:, 64:128]).then_inc(sv, 1)
        nc.vector.wait_ge(spe, 3)
        nc.vector.tensor_copy(s3[:, 0:64], p3[:, 0:64]).then_inc(sv, 1)
        nc.vector.tensor_copy(s3[:, 64:128], p3[:, 64:128]).then_inc(sv, 1)
        nc.vector.wait_ge(spe, 4)
        nc.vector.tensor_copy(s4[:], p4[:]).then_inc(sv, 1)

        # output on the idle scalar ring; no engine waits for completion --
        # the NEFF fini sem-clear sequence overlaps the transfer and NRT
        # drains the DMA queues before execution completes
        nc.scalar.wait_ge(sv, 7)
        nc.scalar.dma_start(o[:], s4[:]).then_inc(so, 16)

    # Hoist the input DMAs ahead of the preamble all-engine barrier so the
    # transfers overlap it (the barrier was emitted at Bass() init, before
    # any user instruction).
    main = nc.m.functions[0].blocks[0]
    il = main.instructions
    for pos, bi in enumerate(hoist, start=1):
        inst = bi.ins
        il.remove(inst)
        il.insert(pos, inst)


    nc.compile()
    _PROG["nc"] = nc
    return nc


def _get_program():
    if _RAW:
        return _get_program_raw()
    if "nc" in _PROG:
        return _PROG["nc"]

    import concourse.bacc as bacc
    import concourse.mybir as mybir
    import concourse.tile as tile

    f32 = mybir.dt.float32
    dt = mybir.dt.float16 if _FP16 else mybir.dt.float32
    # blob cols: [A0 | M1T | W10T | W11T | CreT | CimT | nCimT | W20T | W21T]
    NW = 9
    BLOB = NW * 128

    nc = bacc.Bacc("TRN2", target_bir_lowering=False)
    blob = nc.declare_dram_parameter("blob", [128, BLOB], dt, isOutput=False)
    o = nc.declare_dram_parameter("o", [128, 128], dt, isOutput=True)

    with tile.TileContext(nc) as tc:
        with (
            tc.tile_pool(name="bp", bufs=1) as bp,
            tc.tile_pool(name="ps", bufs=4, space="PSUM") as ps,
            tc.tile_pool(name="sp", bufs=4) as sp,
        ):
            b_sb = bp.tile([128, BLOB], dt)
            nc.sync.dma_start(b_sb[:], blob[:])
            sl = lambda k: b_sb[:, k * 128 : (k + 1) * 128]
            A0, M1T, W10T, W11T, CreT, CimT, nCimT, W20T, W21T = (
                sl(k) for k in range(9)
            )

            p1 = ps.tile([128, 128], f32, tag="p")
            nc.tensor.matmul(p1[:], A0, M1T, start=True, stop=True)
            s1 = sp.tile([128, 128], dt, tag="s")
            nc.vector.tensor_copy(s1[:], p1[:])

            p2 = ps.tile([128, 128], f32, tag="p")
            nc.tensor.matmul(p2[0:64, :], s1[:, 0:64], W10T, start=True, stop=True)
            nc.tensor.matmul(p2[64:128, :], s1[:, 64:128], W11T, start=True, stop=True)
            s2 = sp.tile([128, 128], dt, tag="s")
            nc.vector.tensor_copy(s2[:], p2[:])

            p3 = ps.tile([128, 128], f32, tag="p")
            nc.tensor.matmul(p3[0:64, :], s2[:, 0:64], CreT, start=True, stop=False)
            nc.tensor.matmul(p3[0:64, :], s2[:, 64:128], nCimT, start=False, stop=True)
            nc.tensor.matmul(p3[64:128, :], s2[:, 0:64], CimT, start=True, stop=False)
            nc.tensor.matmul(p3[64:128, :], s2[:, 64:128], CreT, start=False, stop=True)
            s3 = sp.tile([128, 128], dt, tag="s")
            nc.vector.tensor_copy(s3[:], p3[:])

            p4 = ps.tile([128, 128], f32, tag="p")
            nc.tensor.matmul(p4[0:64, :], s3[:, 0:64], W20T, start=True, stop=True)
            nc.tensor.matmul(p4[64:128, :], s3[:, 64:128], W21T, start=True, stop=True)
            s4 = sp.tile([128, 128], f32, tag="s")
            nc.vector.tensor_copy(s4[:], p4[:])

            nc.sync.dma_start(o[:], s4[:])

    nc.compile()
    _PROG["nc"] = nc
    return nc


def kernel(x_real, x_imag, angles, dim, wires):
    from concourse.bass_utils import run_bass_kernel_spmd

    assert int(dim) == 2 and int(wires) == _N
    x_real = np.asarray(x_real, dtype=np.float32)
    x_imag = np.asarray(x_imag, dtype=np.float32)

    W = np.concatenate(_build_weights(angles), axis=1)  # (128, 8*128)

    in_maps = []
    for c in range(_NC):
        cols = slice(4 * c, 4 * c + 4)
        # A0[m, reim*64 + mu*4 + b] = x[m*16+mu, 4c+b]
        a_re = x_real[:, cols].reshape(128, 64)
        a_im = x_imag[:, cols].reshape(128, 64)
        blob = np.concatenate([a_re, a_im, W], axis=1)
        if _FP16:
            blob = blob.astype(np.float16)
        in_maps.append({"blob": np.ascontiguousarray(blob)})

    nc = _get_program()
    res = run_bass_kernel_spmd(nc, in_maps, list(range(_NC)))
    _LAST["res"] = res

    inv = _get_const()["inv"]
    out = np.empty((_D, _B), dtype=np.complex64)
    for c in range(_NC):
        od = res.results[c]["o"].astype(np.float32)  # (128=m'', 128=rho)
        z = od[:, 0:64].reshape(128, 16, 4) + 1j * od[:, 64:128].reshape(128, 16, 4)
        full = np.empty((128, 16, 4), dtype=np.complex64)
        full[inv] = z  # basis m' = inv[m'']
        out[:, 4 * c : 4 * c + 4] = full.reshape(_D, 4)
    return out


# revision 32
# speedup vs baseline: 1.0950x; 1.0950x over previous
"""Trainium2 kernel for nn_Circuit_88871463288913.

Circuit (d=2, n=11 wires, D=2048, B=32):
  psi -> H on every wire -> CNOT ladder -> CRX ladder.

Decomposition (bits q0..q10, q10 fastest; major m = q0..q6, minor mu =
q7..q10):
  - H^11 = H7(major) x H4(minor)
  - CNOT ladder = prefix-XOR permutation = PX7(major), then minor map
    mu' = PX4(mu) ^ (q6' ? 1111 : 0)  conditioned on the post-PX7 bit q6'
  - CRX 0..5 act on major only (complex 128x128 product C)
  - CRX 6 = R_g on q7 conditioned on major bit q6' (g)
  - CRX 7..9 act on minor only (complex 16x16 product K)

Device pipeline per core (batch shard: 4 columns/core).  The state is one
128x128 f32 tile; every gate is a PE matmul with the STATE as the
stationary operand (lhsT), so each matmul applies the gate AND flips the
layout between (major | rho) and (rho | major), rho = reim*64 + mu*4 + b.
A bit-rotation relabel (Prot: m' -> m'' = q6'*64 + rest) makes the q6'
conditioning contiguous halves.

  MM1: rhs = (Prot P7 H7)^T                     -> (rho, m'')
  MM2: rhs = (F_h H4 (x) I4 (x) I2)^T, per half -> (m'', rho)
  MM3: complex C via PSUM accumulation with [Cre | -Cim] stacking
                                                -> (rho, m'')
  MM4: rhs = packed-complex (K R_g)^T, per half -> (m'', rho)

Everything angle-dependent is composed on the host in O(128^2).
"""

import numpy as np

_D = 2048
_N = 11
_B = 32
_NC = 8

_LAST = {}  # stash of the most recent BassKernelResults (for test harness)
_PROG = {}  # cached Bass program
_CONST = {}  # cached angle-independent matrices
_FP16 = True  # device dtype for state/weights (PSUM accumulate is always f32)
_RAW = True  # raw-bass program (no TileContext barrier machinery)


def _walsh(nbits):
    n = 1 << nbits
    i = np.arange(n)
    v = i[:, None] & i[None, :]
    v = v ^ (v >> 8)
    v = v ^ (v >> 4)
    v = v ^ (v >> 2)
    v = v ^ (v >> 1)
    return (1.0 - 2.0 * (v & 1)) / np.sqrt(n)


def _prefix_xor(nbits):
    v = np.arange(1 << nbits)
    res = np.zeros_like(v)
    acc = np.zeros_like(v)
    for k in range(nbits):
        bit = (v >> (nbits - 1 - k)) & 1
        acc = acc ^ bit
        res = res | (acc << (nbits - 1 - k))
    return res


def _perm_mat(p):
    n = len(p)
    M = np.zeros((n, n))
    M[p, np.arange(n)] = 1.0
    return M


def _crx(nbits, cpos, tpos, theta):
    n = 1 << nbits
    i = np.arange(n)
    cd = (i >> (nbits - 1 - cpos)) & 1
    a = cd * theta / 2.0
    flip = i ^ (1 << (nbits - 1 - tpos))
    M = np.zeros((n, n), dtype=complex)
    M[i, i] = np.cos(a)
    M[flip, i] = -1j * np.sin(a)
    return M


def _expand_mu(Mmu):  # 16x16 -> 64x64 (tensor with I4 over batch)
    return np.kron(Mmu, np.eye(4))


def _get_const():
    if _CONST:
        return _CONST
    m = np.arange(128)
    rot = ((m & 1) << 6) | (m >> 1)  # m' -> m''
    inv = np.empty(128, dtype=int)
    inv[rot] = m
    Prot = _perm_mat(rot)
    M1 = Prot @ _perm_mat(_prefix_xor(7)) @ _walsh(7)
    PX4 = _prefix_xor(4)
    H4 = _walsh(4)
    G = [_perm_mat(PX4 ^ (15 * h)) @ H4 for h in (0, 1)]
    W1 = [np.kron(np.eye(2), _expand_mu(G[h])) for h in (0, 1)]
    _CONST.update(rot=rot, inv=inv, Prot=Prot, M1=M1, W1=W1)
    return _CONST


def _build_weights(angles):
    """Returns the 8 rhs matrices (f32, 128x128 each), already transposed."""
    cst = _get_const()
    ang = np.asarray(angles, dtype=np.float64)

    C = np.eye(128, dtype=complex)
    for q in range(6):
        C = _crx(7, q, q + 1, ang[q]) @ C
    Cpp = cst["Prot"] @ C @ cst["Prot"].T

    K = np.eye(16, dtype=complex)
    for q in range(7, 10):
        K = _crx(4, q - 7, q - 6, ang[q]) @ K
    W2 = []
    for g in (0, 1):
        i = np.arange(16)
        a = g * ang[6] / 2.0
        R = np.zeros((16, 16), dtype=complex)
        R[i, i] = np.cos(a)
        R[i ^ 8, i] = -1j * np.sin(a)
        Kg = K @ R
        Kr, Ki = _expand_mu(np.real(Kg)), _expand_mu(np.imag(Kg))
        W2.append(np.block([[Kr, -Ki], [Ki, Kr]]))

    Cre, Cim = np.real(Cpp), np.imag(Cpp)
    mats = [
        cst["M1"].T,
        cst["W1"][0].T,
        cst["W1"][1].T,
        Cre.T,
        Cim.T,
        (-Cim).T,
        W2[0].T,
        W2[1].T,
    ]
    return [np.ascontiguousarray(x, dtype=np.float32) for x in mats]


def _get_program_raw():
    """Raw-bass (no TileContext) version: manual semaphores, no tail
    barrier machinery.  Dataflow is a short serial chain, so manual sync
    is tractable: PE waits on DVE copies, DVE waits on PE matmuls."""
    if "nc" in _PROG:
        return _PROG["nc"]

    import concourse.bacc as bacc
    import concourse.mybir as mybir

    f32 = mybir.dt.float32
    dt = mybir.dt.float16 if _FP16 else mybir.dt.float32
    NW = 9
    BLOB = NW * 128

    nc = bacc.Bacc("TRN2", target_bir_lowering=False)
    blob = nc.declare_dram_parameter("blob", [128, BLOB], dt, isOutput=False)
    o = nc.declare_dram_parameter("o", [128, 128], dt, isOutput=True)

    with (
        nc.sbuf_tensor([128, BLOB], dt) as b_sb,
        nc.sbuf_tensor([128, 128], dt) as s1,
        nc.sbuf_tensor([128, 128], dt) as s2,
        nc.sbuf_tensor([128, 128], dt) as s3,
        nc.sbuf_tensor([128, 128], dt) as s4,
        nc.psum_tensor([128, 128], f32) as p1,
        nc.psum_tensor([128, 128], f32) as p2,
        nc.psum_tensor([128, 128], f32) as p3,
        nc.psum_tensor([128, 128], f32) as p4,
        nc.semaphore() as sd,
        nc.semaphore() as sa,
        nc.semaphore() as sg,
        nc.semaphore() as sh,
        nc.semaphore() as so,
        nc.semaphore() as spe,
        nc.semaphore() as sv,
    ):
        sl = lambda k: b_sb[:, k * 128 : (k + 1) * 128]
        A0, M1T, W10T, W11T, CreT, CimT, nCimT, W20T, W21T = (
            sl(k) for k in range(9)
        )

        # No nc.Block(): instructions are emitted straight into `main`.
        # SWDGE (gpsimd) coalesces packets ~10x better than the HW-DGE
        # rings for these shapes, so all input phases go through it, each
        # with its own semaphore (SW queues may complete out of order).
        hoist = [
            nc.sync.dma_start(b_sb[:, 0:128], blob[:, 0:128]).then_inc(sd, 16),
            nc.scalar.dma_start(b_sb[:, 128:256], blob[:, 128:256]).then_inc(sd, 16),
            nc.sync.dma_start(b_sb[:, 512:896], blob[:, 512:896]).then_inc(sg, 16),
            nc.scalar.dma_start(b_sb[:, 256:512], blob[:, 256:512]).then_inc(sa, 16),
            nc.scalar.dma_start(b_sb[:, 896:1152], blob[:, 896:1152]).then_inc(sh, 16),
        ]

        # PE chain
        nc.tensor.wait_ge(sd, 32)
        nc.tensor.matmul(p1[:], A0, M1T, start=True, stop=True).then_inc(spe, 1)
        nc.tensor.wait_ge(sa, 16)
        nc.tensor.wait_ge(sv, 1)
        nc.tensor.matmul(p2[0:64, :], s1[:, 0:64], W10T, start=True, stop=True)
        nc.tensor.wait_ge(sv, 2)
        nc.tensor.matmul(
            p2[64:128, :], s1[:, 64:128], W11T, start=True, stop=True
        ).then_inc(spe, 1)
        nc.tensor.wait_ge(sg, 16)
        nc.tensor.wait_ge(sv, 3)
        nc.tensor.matmul(p3[0:64, :], s2[:, 0:64], CreT, start=True, stop=False)
        nc.tensor.wait_ge(sv, 4)
        nc.tensor.matmul(p3[0:64, :], s2[:, 64:128], nCimT, start=False, stop=True)
        nc.tensor.matmul(p3[64:128, :], s2[:, 0:64], CimT, start=True, stop=False)
        nc.tensor.matmul(
            p3[64:128, :], s2[:, 64:128], CreT, start=False, stop=True
        ).then_inc(spe, 1)
        nc.tensor.wait_ge(sh, 16)
        nc.tensor.wait_ge(sv, 5)
        nc.tensor.matmul(p4[0:64, :], s3[:, 0:64], W20T, start=True, stop=True)
        nc.tensor.wait_ge(sv, 6)
        nc.tensor.matmul(
            p4[64:128, :], s3[:, 64:128], W21T, start=True, stop=True
        ).then_inc(spe, 1)

        # DVE chain (PSUM -> SBUF bounces), split in free-dim halves so the
        # next PE stage starts as soon as its lhsT half is converted
        nc.vector.wait_ge(spe, 1)
        nc.vector.tensor_copy(s1[:, 0:64], p1[:, 0:64]).then_inc(sv, 1)
        nc.vector.tensor_copy(s1[:, 64:128], p1[:, 64:128]).then_inc(sv, 1)
        nc.vector.wait_ge(spe, 2)
        nc.vector.tensor_copy(s2[:, 0:64], p2[:, 0:64]).then_inc(sv, 1)
        nc.vector.tensor_copy(s2[:, 64:128], p2[:, 64:128]).then_inc(sv, 1)
        nc.vector.wait_ge(spe, 3)
        nc.vector.tensor_copy(s3[:, 0:64], p3[:, 0:64]).then_inc(sv, 1)
        nc.vector.tensor_copy(s3[:, 64:128], p3[:, 64:128]).then_inc(sv, 1)
        nc.vector.wait_ge(spe, 4)
        nc.vector.tensor_copy(s4[:], p4[:]).then_inc(sv, 1)

        # output on the idle scalar ring; no engine waits for completion --
        # the NEFF fini sem-clear sequence overlaps the transfer and NRT
        # drains the DMA queues before execution completes
        nc.scalar.wait_ge(sv, 7)
        nc.scalar.dma_start(o[:], s4[:]).then_inc(so, 16)

    # Hoist the input DMAs ahead of the preamble all-engine barrier so the
    # transfers overlap it (the barrier was emitted at Bass() init, before
    # any user instruction).
    main = nc.m.functions[0].blocks[0]
    il = main.instructions
    for pos, bi in enumerate(hoist, start=1):
        inst = bi.ins
        il.remove(inst)
        il.insert(pos, inst)


    nc.compile()
    _PROG["nc"] = nc
    return nc


def _get_program():
    if _RAW:
        return _get_program_raw()
    if "nc" in _PROG:
        return _PROG["nc"]

    import concourse.bacc as bacc
    import concourse.mybir as mybir
    import concourse.tile as tile

    f32 = mybir.dt.float32
    dt = mybir.dt.float16 if _FP16 else mybir.dt.float32
    # blob cols: [A0 | M1T | W10T | W11T | CreT | CimT | nCimT | W20T | W21T]
    NW = 9
    BLOB = NW * 128

    nc = bacc.Bacc("TRN2", target_bir_lowering=False)
    blob = nc.declare_dram_parameter("blob", [128, BLOB], dt, isOutput=False)
    o = nc.declare_dram_parameter("o", [128, 128], dt, isOutput=True)

    with tile.TileContext(nc) as tc:
        with (
            tc.tile_pool(name="bp", bufs=1) as bp,
            tc.tile_pool(name="ps", bufs=4, space="PSUM") as ps,
            tc.tile_pool(name="sp", bufs=4) as sp,
        ):
            b_sb = bp.tile([128, BLOB], dt)
            nc.sync.dma_start(b_sb[:], blob[:])
            sl = lambda k: b_sb[:, k * 128 : (k + 1) * 128]
            A0, M1T, W10T, W11T, CreT, CimT, nCimT, W20T, W21T = (
                sl(k) for k in range(9)
            )

            p1 = ps.tile([128, 128], f32, tag="p")
            nc.tensor.matmul(p1[:], A0, M1T, start=True, stop=True)
            s1 = sp.tile([128, 128], dt, tag="s")
            nc.vector.tensor_copy(s1[:], p1[:])

            p2 = ps.tile([128, 128], f32, tag="p")
            nc.tensor.matmul(p2[0:64, :], s1[:, 0:64], W10T, start=True, stop=True)
            nc.tensor.matmul(p2[64:128, :], s1[:, 64:128], W11T, start=True, stop=True)
            s2 = sp.tile([128, 128], dt, tag="s")
            nc.vector.tensor_copy(s2[:], p2[:])

            p3 = ps.tile([128, 128], f32, tag="p")
            nc.tensor.matmul(p3[0:64, :], s2[:, 0:64], CreT, start=True, stop=False)
            nc.tensor.matmul(p3[0:64, :], s2[:, 64:128], nCimT, start=False, stop=True)
            nc.tensor.matmul(p3[64:128, :], s2[:, 0:64], CimT, start=True, stop=False)
            nc.tensor.matmul(p3[64:128, :], s2[:, 64:128], CreT, start=False, stop=True)
            s3 = sp.tile([128, 128], dt, tag="s")
            nc.vector.tensor_copy(s3[:], p3[:])

            p4 = ps.tile([128, 128], f32, tag="p")
            nc.tensor.matmul(p4[0:64, :], s3[:, 0:64], W20T, start=True, stop=True)
            nc.tensor.matmul(p4[64:128, :], s3[:, 64:128], W21T, start=True, stop=True)
            s4 = sp.tile([128, 128], f32, tag="s")
            nc.vector.tensor_copy(s4[:], p4[:])

            nc.sync.dma_start(o[:], s4[:])

    nc.compile()
    _PROG["nc"] = nc
    return nc


def kernel(x_real, x_imag, angles, dim, wires):
    from concourse.bass_utils import run_bass_kernel_spmd

    assert int(dim) == 2 and int(wires) == _N
    x_real = np.asarray(x_real, dtype=np.float32)
    x_imag = np.asarray(x_imag, dtype=np.float32)

    W = np.concatenate(_build_weights(angles), axis=1)  # (128, 8*128)

    in_maps = []
    for c in range(_NC):
        cols = slice(4 * c, 4 * c + 4)
        # A0[m, reim*64 + mu*4 + b] = x[m*16+mu, 4c+b]
        a_re = x_real[:, cols].reshape(128, 64)
        a_im = x_imag[:, cols].reshape(128, 64)
        blob = np.concatenate([a_re, a_im, W], axis=1)
        if _FP16:
            blob = blob.astype(np.float16)
        in_maps.append({"blob": np.ascontiguousarray(blob)})

    nc = _get_program()
    res = run_bass_kernel_spmd(nc, in_maps, list(range(_NC)))
    _LAST["res"] = res

    inv = _get_const()["inv"]
    out = np.empty((_D, _B), dtype=np.complex64)
    for c in range(_NC):
        od = res.results[c]["o"].astype(np.float32)  # (128=m'', 128=rho)
        z = od[:, 0:64].reshape(128, 16, 4) + 1j * od[:, 64:128].reshape(128, 16, 4)
        full = np.empty((128, 16, 4), dtype=np.complex64)
        full[inv] = z  # basis m' = inv[m'']
        out[:, 4 * c : 4 * c + 4] = full.reshape(_D, 4)
    return out
